# revision 1
# baseline (speedup 1.0000x reference)
"""Trainium2 Bass kernel: Tacotron-style location-sensitive attention step.

Sharding strategy (8 NeuronCores, SPMD):
  - Attention / conv / softmax / context: pure batch parallelism
    (B=128 -> 16 examples per core; enc_seq, proc_mem, attention weights,
    mask sharded on the batch dim host-side).
  - LSTM cell: H-sharded. Core j computes h.T rows [128j, 128j+128) for the
    FULL batch using only 1/8 of W_ih/W_hh (3.5 MB instead of 28 MB of
    replicated weight traffic), then a small AllGather of h.T.
    qry2 = h @ W_q.T + const is computed for the full batch and each core
    selects its 16 rows with a one-hot matmul (bsel input) so the SPMD
    graph stays core-uniform.

Compute dtypes: bf16 operands for all large matmuls / elementwise (well
inside the 2e-2 rel-err budget), f32 for PSUM, softmax and reductions.

kernel(**inputs) takes FULL numpy inputs (as produced by setup_inputs())
and returns the FULL [128, 512] float32 context.
"""

import sys

sys.path.insert(0, "/opt/trn_rl_repo")

import numpy as np

import concourse.bass as bass
import concourse.mybir as mybir
from concourse import bacc
from concourse.bass_utils import run_bass_kernel_spmd
from concourse.masks import make_identity
from concourse.bass import _add_dep_helper
from concourse.tile import TileContext

F32 = mybir.dt.float32
BF16 = mybir.dt.bfloat16
AF = mybir.ActivationFunctionType

B, S, E, P, H, A, F, KW = 128, 1024, 512, 256, 1024, 128, 32, 31
NCORES = 8
BL = B // NCORES        # 16 examples per core
HL = H // NCORES        # 128 h rows per core
PE_DIM = P + E          # 768
NKI = PE_DIM // 128     # 6
NKH = H // 128          # 8
NC_S = S // 128         # 8 s-chunks
PADW = KW // 2          # 15
CONVROW = PADW + S + 17  # 1056 padded per-channel staging row
TAPS = 2 * KW           # 62
ENC_T = 8               # s-chunks per enc DMA tile (whole example)
ENC_BUFS = 10
GRP = 4                 # examples per fused softmax/context group


def build():
    nc = bacc.Bacc("TRN2", target_bir_lowering=False, debug=False,
                   num_devices=NCORES)

    dp = nc.declare_dram_parameter
    prenet = dp("prenet", [B, P], F32, isOutput=False)
    prev_ctx = dp("prev_ctx", [B, E], F32, isOutput=False)
    att_h = dp("att_h", [B, H], F32, isOutput=False)
    att_c_sh = dp("att_c_sh", [B, HL], F32, isOutput=False)
    w_ih_sh = dp("w_ih_sh", [4, HL, PE_DIM], F32, isOutput=False)
    w_hh_sh = dp("w_hh_sh", [4, HL, H], F32, isOutput=False)
    b_ih_sh = dp("b_ih_sh", [4, HL], F32, isOutput=False)
    b_hh_sh = dp("b_hh_sh", [4, HL], F32, isOutput=False)
    prev_w = dp("prev_w", [BL, S], F32, isOutput=False)
    cum_w = dp("cum_w", [BL, S], F32, isOutput=False)
    enc = dp("enc", [BL, S, E], F32, isOutput=False)
    proc = dp("proc", [BL, S, A], F32, isOutput=False)
    conv_w = dp("conv_w", [F, 2, KW], F32, isOutput=False)
    conv_b = dp("conv_b", [F, 1], F32, isOutput=False)
    w_loc = dp("w_loc", [A, F], F32, isOutput=False)
    b_loc = dp("b_loc", [1, A], F32, isOutput=False)
    w_q = dp("w_q", [A, H], F32, isOutput=False)
    b_q = dp("b_q", [1, A], F32, isOutput=False)
    w_out = dp("w_out", [1, A], F32, isOutput=False)
    bsel = dp("bsel", [B, BL], F32, isOutput=False)
    out = dp("out", [BL, E], F32, isOutput=True)

    with TileContext(nc) as tc:
        with (
            tc.tile_pool(name="const", bufs=1) as cpool,
            tc.tile_pool(name="work", bufs=2) as wpool,
            tc.tile_pool(name="xpadp", bufs=6) as xpool,
            tc.tile_pool(name="conv", bufs=16) as convpool,
            tc.tile_pool(name="proc", bufs=16) as ppool,
            tc.tile_pool(name="vbig", bufs=2) as vpool,
            tc.tile_pool(name="psA", bufs=2, space="PSUM") as psA,
            tc.tile_pool(name="psV", bufs=2, space="PSUM") as psV,
            tc.tile_pool(name="psX", bufs=2, space="PSUM") as psX,
            tc.tile_pool(name="dram", bufs=1, space="DRAM") as dpool,
        ):
            def mm_ps(shape):
                t = psA.tile([128, 512], F32, tag="mm")
                return t[: shape[0], : shape[1]]

            # ------------- constants / small preprocessing -------------
            ident = cpool.tile([128, 128], F32)
            make_identity(nc, ident[:])
            id_bf = cpool.tile([128, 128], BF16)
            nc.vector.tensor_copy(id_bf[:], ident[:])
            ones_row = cpool.tile([1, 128], F32)
            nc.vector.memset(ones_row[:], 1.0)

            pe_t_ctr = [0]

            def pe_t(dst, src_ap, rows, engine=None):
                """dst = src_ap([rows, cols]).T via TensorE (+copy/cast)."""
                ps = mm_ps((dst.shape[0], rows))
                nc.tensor.transpose(ps, src_ap, ident[:rows, :rows])
                if engine is None:
                    pe_t_ctr[0] += 1
                    engine = "dve" if pe_t_ctr[0] % 2 else "act"
                if engine == "dve":
                    nc.vector.tensor_copy(dst, ps)
                else:
                    nc.scalar.copy(dst, ps)

            # ---- LSTM weight shard: load FIRST (DMA priority), PE-T, DVE copy
            # Lives in its own pool, closed after the gates so the SBUF is
            # recycled for the enc prefetch pool.
            NK = NKI + NKH  # 14
            wtpool_cm = tc.tile_pool(name="wt", bufs=1)
            wtpool = wtpool_cm.__enter__()
            wT = wtpool.tile([128, 4, NK, HL], BF16)
            wnats = []
            for g in range(4):
                wi_nat = wtpool.tile([HL, PE_DIM], F32, tag=f"wload{g}")
                nc.sync.dma_start(wi_nat[:], w_ih_sh[g])
                wh_nat = wtpool.tile([HL, H], F32, tag=f"wload2{g}")
                nc.sync.dma_start(wh_nat[:], w_hh_sh[g])
                wnats.append((wi_nat, wh_nat))

            # activations for the LSTM (needed right after W)
            pn_nat = wtpool.tile([B, P], F32)
            nc.sync.dma_start(pn_nat[:], prenet[:])
            pc_nat = wtpool.tile([B, E], F32)
            nc.sync.dma_start(pc_nat[:], prev_ctx[:])
            ah_nat = wtpool.tile([B, H], F32)
            nc.sync.dma_start(ah_nat[:], att_h[:])
            ac_nat = wtpool.tile([B, HL], F32)
            nc.sync.dma_start(ac_nat[:], att_c_sh[:])
            bi_nat = wtpool.tile([4, HL], F32)
            nc.sync.dma_start(bi_nat[:], b_ih_sh[:])
            bh_nat = wtpool.tile([4, HL], F32)
            crit_last = nc.sync.dma_start(bh_nat[:], b_hh_sh[:])

            def pe_t_multi(dst_ap, srcs, rows, engine):
                """Transpose several 128-col chunks into one psA tile, then
                copy them out with a single wide copy."""
                ps = psA.tile([128, 512], F32, tag="mm")
                for i, s_ap in enumerate(srcs):
                    nc.tensor.transpose(ps[:, i * rows:(i + 1) * rows], s_ap,
                                        ident[:rows, :rows])
                if engine == "dve":
                    nc.vector.tensor_copy(dst_ap, ps[:, :len(srcs) * rows])
                else:
                    nc.scalar.copy(dst_ap, ps[:, :len(srcs) * rows])

            for g in range(4):
                wi_nat, wh_nat = wnats[g]
                chunks = [wi_nat[:, k * 128:(k + 1) * 128] for k in range(NKI)]
                chunks += [wh_nat[:, k * 128:(k + 1) * 128] for k in range(NKH)]
                NK2 = NKI + NKH
                for q in range(0, NK2, 4):
                    qs = chunks[q:q + 4]
                    pe_t_multi(wT[:, g, q:q + len(qs), :], qs, HL,
                               "dve" if (q // 4) % 2 else "act")

            inpT = cpool.tile([128, NKI, B], BF16)
            ichunks = [pn_nat[:, k * 128:(k + 1) * 128] for k in range(2)]
            ichunks += [pc_nat[:, k * 128:(k + 1) * 128] for k in range(4)]
            pe_t_multi(inpT[:, 0:4, :], ichunks[0:4], B, "act")
            pe_t_multi(inpT[:, 4:6, :], ichunks[4:6], B, "dve")
            ahT = cpool.tile([128, NKH, B], BF16)
            achunks = [ah_nat[:, k * 128:(k + 1) * 128] for k in range(NKH)]
            pe_t_multi(ahT[:, 0:4, :], achunks[0:4], B, "act")
            pe_t_multi(ahT[:, 4:8, :], achunks[4:8], B, "dve")
            acT = cpool.tile([HL, B], BF16)
            pe_t(acT[:], ac_nat[:], B)
            nc.vector.tensor_add(bi_nat[:], bi_nat[:], bh_nat[:])
            bias_sb = cpool.tile([HL, 4], F32)
            pe_t(bias_sb[:], bi_nat[:], 4)

            # ---- gates (H-shard, full batch) -> h.T shard, as early as possible
            gate_sb = []
            for g in range(4):
                ps = mm_ps((HL, B))
                for k in range(NKI):
                    nc.tensor.matmul(ps, wT[:, g, k, :], inpT[:, k, :],
                                     start=(k == 0), stop=False)
                for k in range(NKH):
                    nc.tensor.matmul(ps, wT[:, g, NKI + k, :], ahT[:, k, :],
                                     start=False, stop=(k == NKH - 1))
                sb = cpool.tile([HL, B], BF16, tag=f"gate{g}")
                fn = AF.Tanh if g == 2 else AF.Sigmoid
                nc.scalar.activation(sb[:], ps, fn, bias=bias_sb[:, g:g + 1])
                gate_sb.append(sb)

            cT = cpool.tile([HL, B], BF16)
            nc.vector.tensor_mul(cT[:], gate_sb[1][:], acT[:])
            tg = cpool.tile([HL, B], BF16)
            nc.vector.tensor_mul(tg[:], gate_sb[0][:], gate_sb[2][:])
            nc.vector.tensor_add(cT[:], cT[:], tg[:])
            nc.scalar.activation(tg[:], cT[:], AF.Tanh)
            hT_sh = cpool.tile([HL, B], BF16)
            nc.vector.tensor_mul(hT_sh[:], gate_sb[3][:], tg[:])
            h_in = dpool.tile([HL, B], BF16)
            nc.scalar.dma_start(h_in[:], hT_sh[:])
            wtpool_cm.__exit__(None, None, None)
            epool_cm = tc.tile_pool(name="enc", bufs=ENC_BUFS)
            epool = epool_cm.__enter__()

            # ---- small constant preprocessing (off the critical path)
            cw_nat = cpool.tile([F, TAPS], F32)
            nc.sync.dma_start(cw_nat[:], conv_w.rearrange("f c k -> f (c k)"))
            w2 = cpool.tile([TAPS, F], BF16)
            pe_t(w2[:], cw_nat[:], F)

            wl_nat = cpool.tile([A, F], F32)
            nc.sync.dma_start(wl_nat[:], w_loc[:])
            wlocT = cpool.tile([F, A], F32)
            pe_t(wlocT[:], wl_nat[:], A)

            cb_col = cpool.tile([F, 1], F32)
            nc.sync.dma_start(cb_col[:], conv_b[:])
            bl_row = cpool.tile([1, A], F32)
            nc.sync.dma_start(bl_row[:], b_loc[:])
            bq_row = cpool.tile([1, A], F32)
            nc.sync.dma_start(bq_row[:], b_q[:])
            ps = mm_ps((1, A))
            nc.tensor.matmul(ps, cb_col[:], wlocT[:], start=True, stop=True)
            const_row = cpool.tile([1, A], F32)
            nc.vector.tensor_add(const_row[:], ps, bl_row[:])
            nc.vector.tensor_add(const_row[:], const_row[:], bq_row[:])

            wo_row = cpool.tile([1, A], F32)
            nc.sync.dma_start(wo_row[:], w_out[:])
            ps = mm_ps((128, A))
            nc.tensor.matmul(ps, ones_row[:], wo_row[:], start=True, stop=True)
            wo_rep8 = cpool.tile([128, NC_S, A], BF16)
            for c in range(NC_S):
                nc.scalar.copy(wo_rep8[:, c, :], ps)

            sel_sb = cpool.tile([B, BL], F32)
            nc.sync.dma_start(sel_sb[:], bsel[:])


            wq_nat = cpool.tile([A, H], F32)
            nc.sync.dma_start(wq_nat[:], w_q[:])
            wqT = cpool.tile([128, NKH, A], BF16)
            qchunks = [wq_nat[:, k * 128:(k + 1) * 128] for k in range(NKH)]
            pe_t_multi(wqT[:, 0:4, :], qchunks[0:4], A, "act")
            pe_t_multi(wqT[:, 4:8, :], qchunks[4:8], A, "dve")

            # padded conv input rows staged to DRAM (bf16):
            # row layout per (b, c): [15 zeros | 1024 data | 17 zeros]
            stage = cpool.tile([BL, 2 * CONVROW], BF16)
            nc.vector.memset(stage[:], 0.0)
            nc.gpsimd.dma_start(stage[:, PADW:PADW + S], cum_w[:])
            nc.gpsimd.dma_start(stage[:, CONVROW + PADW:CONVROW + PADW + S],
                                prev_w[:])
            pad_dram = dpool.tile([BL, 2 * CONVROW], BF16)
            nc.sync.dma_start(pad_dram[:], stage[:])
            # materialize all 62 overlapping window rows per example in DRAM
            win_dram = dpool.tile([BL, TAPS, S], BF16)
            for c in range(2):
                sb2 = pad_dram[0, c * CONVROW:c * CONVROW + 1]
                wsrc = bass.AP(
                    tensor=sb2.tensor,
                    offset=sb2.offset,
                    ap=[[2 * CONVROW, BL], [1, KW], [1, S]],
                )
                db2 = win_dram[0, c * KW:c * KW + 1, 0:1]
                wdst = bass.AP(
                    tensor=db2.tensor,
                    offset=db2.offset,
                    ap=[[TAPS * S, BL], [S, KW], [1, S]],
                )
                nc.sync.dma_start(wdst, wsrc)

            # ---- streaming preloads (bf16 casts on the gpsimd queue)
            proc_tiles = []
            for b in range(6):
                pt = ppool.tile([128, NC_S, A], BF16, tag="proc")
                pdma = nc.gpsimd.dma_start(
                    pt[:], proc[b].rearrange("(p r) a -> p r a", r=NC_S))
                if b == 0:
                    _add_dep_helper(pdma.ins, crit_last.ins, sync=True,
                                    reason="preloads yield DMA BW to LSTM-critical loads")
                proc_tiles.append(pt)
            # ---- location conv (contiguous per-example window loads)
            conv_tiles = []
            for b in range(BL):
                xpadT = xpool.tile([TAPS, S], BF16, tag="xpad")
                nc.sync.dma_start(xpadT[:], win_dram[b])
                conv_sb = convpool.tile([F + 1, S], BF16, tag="conv")
                for h2 in range(2):
                    ps = mm_ps((F, 512))
                    nc.tensor.matmul(ps, w2[:],
                                     xpadT[:, h2 * 512:(h2 + 1) * 512],
                                     start=True, stop=True)
                    nc.scalar.copy(
                        conv_sb[:F, h2 * 512:(h2 + 1) * 512], ps)
                nc.vector.memset(conv_sb[F:F + 1, :], 1.0)
                conv_tiles.append(conv_sb)

            # ---- AllGather h.T (fires as soon as h_in lands)
            h_gat = dpool.tile([NCORES, HL, B], BF16)
            nc.gpsimd.collective_compute(
                "AllGather",
                mybir.AluOpType.bypass,
                replica_groups=[list(range(NCORES))],
                ins=[h_in[:].opt()],
                outs=[h_gat[:].opt()],
            )

            # remaining streams on gpsimd AFTER the collective: their slot
            # stalls resolve through sync/PE/DVE work only (deadlock-safe)
            for b in range(6, BL):
                pt = ppool.tile([128, NC_S, A], BF16, tag="proc")
                nc.gpsimd.dma_start(
                    pt[:], proc[b].rearrange("(p r) a -> p r a", r=NC_S))
                proc_tiles.append(pt)

            enc_tiles = []
            for b in range(10):
                et = epool.tile([128, ENC_T, E], BF16, tag="enc")
                nc.gpsimd.dma_start(
                    et[:], enc[b].rearrange("(p r) e -> p r e", r=NC_S))
                enc_tiles.append(et)


            for b in range(10, BL):
                et = epool.tile([128, ENC_T, E], BF16, tag="enc")
                nc.gpsimd.dma_start(
                    et[:], enc[b].rearrange("(p r) e -> p r e", r=NC_S))
                enc_tiles.append(et)

            hfull = cpool.tile([128, NKH, B], BF16)
            nc.scalar.dma_start(hfull[:], h_gat[:].rearrange("c p b -> p c b"))

            # ---- qry2 (full batch) + batch selection
            ps_q = mm_ps((B, A))
            for k in range(NKH):
                nc.tensor.matmul(ps_q, hfull[:, k, :], wqT[:, k, :],
                                 start=(k == 0), stop=False)
            nc.tensor.matmul(ps_q, ones_row[:], const_row[:],
                             start=False, stop=True)
            qry2_all = cpool.tile([B, A], F32)
            nc.vector.tensor_copy(qry2_all[:], ps_q)
            ps_q2 = mm_ps((BL, A))
            nc.tensor.matmul(ps_q2, sel_sb[:], qry2_all[:],
                             start=True, stop=True)
            qry2 = cpool.tile([BL, A], BF16)
            nc.vector.tensor_copy(qry2[:], ps_q2)

            # rhs_all[:, b, :] = [W_loc.T ; qry2[b]]  (K=33 fused loc+qry mm)
            rhs_all = cpool.tile([F + 1, BL, A], BF16)
            for b in range(BL):
                nc.vector.tensor_copy(rhs_all[:F, b, :], wlocT[:])
            qdram = dpool.tile([BL, A], BF16)
            nc.scalar.dma_start(qdram[:], qry2[:])
            qsrc = bass.AP(
                tensor=qdram[:].tensor,
                offset=qdram[:].offset,
                ap=[[BL * A, 1], [A, BL], [1, A]],
            )
            nc.scalar.dma_start(rhs_all[F:F + 1, :, :], qsrc)

            # ---- fused tail: scores -> group softmax -> context, streaming
            scoresT = cpool.tile([128, NC_S, BL], F32)
            wTt = cpool.tile([128, NC_S, BL], BF16)
            for g in range(BL // GRP):
                bs = range(g * GRP, (g + 1) * GRP)
                for b in bs:
                    conv_sb = conv_tiles[b]
                    ps_v = psV.tile([128, NC_S * A], F32, tag="v")
                    for c in range(NC_S):
                        nc.tensor.matmul(
                            ps_v[:, c * A:(c + 1) * A],
                            conv_sb[:, c:S:NC_S],
                            rhs_all[:, b, :],
                            start=True, stop=True)
                    v_sb = vpool.tile([128, NC_S, A], BF16, tag="v_sb")
                    nc.vector.tensor_add(
                        v_sb[:],
                        ps_v[:].rearrange("p (c a) -> p c a", c=NC_S),
                        proc_tiles[b][:])
                    nc.scalar.activation(v_sb[:], v_sb[:], AF.Tanh)
                    nc.vector.tensor_mul(v_sb[:], v_sb[:], wo_rep8[:])
                    nc.vector.reduce_sum(scoresT[:, :, b], v_sb[:],
                                         axis=mybir.AxisListType.X)

                # group softmax over S in [b, s] layout
                sc = wpool.tile([GRP, S], F32, tag="scg")
                for c in range(NC_S):
                    pe_t(sc[:, c * 128:(c + 1) * 128],
                         scoresT[:, c, g * GRP:(g + 1) * GRP], 128,
                         engine="act")
                mx = wpool.tile([GRP, 1], F32, tag="mxg")
                nc.vector.reduce_max(mx[:], sc[:], axis=mybir.AxisListType.X)
                nc.vector.tensor_scalar_mul(mx[:], mx[:], -1.0)
                sums = wpool.tile([GRP, 1], F32, tag="smg")
                nc.scalar.activation(sc[:], sc[:], AF.Exp, bias=mx[:],
                                     accum_out=sums[:])
                rs = wpool.tile([GRP, 1], F32, tag="rsg")
                nc.vector.reciprocal(rs[:], sums[:])
                nc.vector.tensor_scalar_mul(sc[:], sc[:], rs[:])
                for c in range(NC_S):
                    pe_t(wTt[:, c, g * GRP:(g + 1) * GRP],
                         sc[:, c * 128:(c + 1) * 128], GRP, engine="act")

                # context for this group
                for b in bs:
                    ps_x = psX.tile([1, E], F32, tag="ctx")
                    for c in range(NC_S):
                        nc.tensor.matmul(ps_x, wTt[:, c, b:b + 1],
                                         enc_tiles[b][:, c, :],
                                         start=(c == 0), stop=(c == NC_S - 1))
                    ctx_row = wpool.tile([1, E], F32, tag="ctxrow")
                    nc.scalar.copy(ctx_row[:], ps_x)
                    nc.sync.dma_start(out[b:b + 1, :], ctx_row[:])

            epool_cm.__exit__(None, None, None)

    nc.compile()
    return nc


_NC_CACHE = None


def _get_nc():
    global _NC_CACHE
    if _NC_CACHE is None:
        _NC_CACHE = build()
    return _NC_CACHE


def shard_inputs(prenet, prev_context, att_h, att_c, prev_weights, cum_weights,
                 enc_seq, proc_mem, mask, W_ih, W_hh, b_ih, b_hh, conv_w,
                 conv_b, W_loc, b_loc, W_q, b_q, W_out, **_unused):
    f = np.ascontiguousarray
    w_ih4 = np.asarray(W_ih, np.float32).reshape(4, H, PE_DIM)
    w_hh4 = np.asarray(W_hh, np.float32).reshape(4, H, H)
    b_ih4 = np.asarray(b_ih, np.float32).reshape(4, H)
    b_hh4 = np.asarray(b_hh, np.float32).reshape(4, H)
    in_maps = []
    for j in range(NCORES):
        bj = slice(BL * j, BL * (j + 1))
        hj = slice(HL * j, HL * (j + 1))
        sel = np.zeros((B, BL), np.float32)
        sel[BL * j:BL * (j + 1), :] = np.eye(BL, dtype=np.float32)
        in_maps.append({
            "prenet": f(np.asarray(prenet, np.float32)),
            "prev_ctx": f(np.asarray(prev_context, np.float32)),
            "att_h": f(np.asarray(att_h, np.float32)),
            "att_c_sh": f(np.asarray(att_c, np.float32)[:, hj]),
            "w_ih_sh": f(w_ih4[:, hj]),
            "w_hh_sh": f(w_hh4[:, hj]),
            "b_ih_sh": f(b_ih4[:, hj]),
            "b_hh_sh": f(b_hh4[:, hj]),
            "prev_w": f(np.asarray(prev_weights, np.float32)[bj]),
            "cum_w": f(np.asarray(cum_weights, np.float32)[bj]),
            "enc": f(np.asarray(enc_seq, np.float32)[bj]),
            "proc": f(np.asarray(proc_mem, np.float32)[bj]),
            "conv_w": f(np.asarray(conv_w, np.float32)),
            "conv_b": f(np.asarray(conv_b, np.float32).reshape(F, 1)),
            "w_loc": f(np.asarray(W_loc, np.float32)),
            "b_loc": f(np.asarray(b_loc, np.float32).reshape(1, A)),
            "w_q": f(np.asarray(W_q, np.float32)),
            "b_q": f(np.asarray(b_q, np.float32).reshape(1, A)),
            "w_out": f(np.asarray(W_out, np.float32).reshape(1, A)),
            "bsel": sel,
        })
    return in_maps


def kernel(**inputs):
    assert not np.any(np.asarray(inputs["mask"])), \
        "kernel assumes mask == 0 (softmax-shift support not implemented)"
    nc = _get_nc()
    in_maps = shard_inputs(**inputs)
    res = run_bass_kernel_spmd(nc, in_maps, core_ids=list(range(NCORES)))
    return np.concatenate([res.results[j]["out"] for j in range(NCORES)],
                          axis=0)


if __name__ == "__main__":
    rng = np.random.default_rng(0)
    print("building...")
    _get_nc()
    print("built ok")



# revision 3
# speedup vs baseline: 1.9332x; 1.9332x over previous
"""Trainium2 Bass kernel: Tacotron-style location-sensitive attention step.

Sharding (8 NeuronCores, SPMD):
  - Batch dim sharded for everything per-example (enc_seq, proc_mem,
    conv windows, softmax, context): 16 examples per core.
  - LSTM cell H-sharded: core j computes hT rows [128j, 128j+128) for the
    FULL batch from 1/8 of W_ih/W_hh, then contributes a partial
    qry2 = h @ W_q.T which is combined with a ReduceScatter (each core
    receives the summed qry2 rows for exactly its 16 examples).

All heavy operands are pre-cast to bf16 AND pre-transposed/pre-windowed on
the host so that every device DMA is a plain (no-cast) HWDGE transfer and
the tensor engine never transposes weights:
  - wt: LSTM weights in [k-part, gate, k-chunk, hl] layout
  - win2: conv input windows (62 taps, zero-padded), 2 examples packed
    per 128-partition tile
  - wcomb: W_loc folded into the conv kernel (the F=32 conv-channel dim
    is contracted away on the host), so location features come out of a
    single K=64 matmul per (example, half)
  - enc: row-permuted so the context matmul s-chunks line up with the
    softmax transpose layout while DMAs stay contiguous per partition
  - procT: per-example transposed to [A, S] so qry2 can be added as a
    per-partition ACT bias inside the tanh

Compute: bf16 operands for matmuls/elementwise, f32 PSUM + softmax.

kernel(**inputs) takes FULL numpy inputs and returns FULL [128, 512] f32
context.
"""

import sys

sys.path.insert(0, "/opt/trn_rl_repo")

import numpy as np
import ml_dtypes

import concourse.bass as bass
import concourse.mybir as mybir
from concourse import bacc
from concourse.bass_utils import run_bass_kernel_spmd
from concourse.masks import make_identity
from concourse.tile import TileContext

F32 = mybir.dt.float32
BF16 = mybir.dt.bfloat16
AF = mybir.ActivationFunctionType
BF = ml_dtypes.bfloat16

B, S, E, P, H, A, F, KW = 128, 1024, 512, 256, 1024, 128, 32, 31
NCORES = 8
BL = B // NCORES        # 16 examples per core
HL = H // NCORES        # 128 h rows per core
PE_DIM = P + E          # 768
NKI = PE_DIM // 128     # 6
NKH = H // 128          # 8
NK = NKI + NKH          # 14
NC_S = S // 128         # 8 s-chunks
TAPS = 2 * KW           # 62
ENC_BUFS = 14


def build():
    nc = bacc.Bacc("TRN2", target_bir_lowering=False, debug=False,
                   num_devices=NCORES)

    dp = nc.declare_dram_parameter
    wt_a = dp("wt_a", [128, 2, NK, HL], BF16, isOutput=False)
    wt_b = dp("wt_b", [128, 2, NK, HL], BF16, isOutput=False)
    xT = dp("xT", [128, NKI, B], BF16, isOutput=False)
    ahT = dp("ahT", [128, NKH, B], BF16, isOutput=False)
    acT = dp("acT", [HL, B], BF16, isOutput=False)
    bias = dp("bias", [HL, 4], F32, isOutput=False)
    wqT = dp("wqT", [HL, A], BF16, isOutput=False)
    const_a = dp("const_a", [A, 1], F32, isOutput=False)
    wo_col = dp("wo_col", [A, 1], BF16, isOutput=False)
    wcomb2 = dp("wcomb2", [128, A], BF16, isOutput=False)
    win2 = dp("win2", [BL // 2, 128, S], BF16, isOutput=False)
    procT = dp("procT", [BL, A, S], BF16, isOutput=False)
    enc = dp("enc", [BL, 128, NC_S * E], BF16, isOutput=False)
    out = dp("out", [BL, E], F32, isOutput=True)

    with TileContext(nc) as tc:
        with (
            tc.tile_pool(name="const", bufs=1) as cpool,
            tc.tile_pool(name="win", bufs=3) as winp,
            tc.tile_pool(name="proc", bufs=6) as procp,
            tc.tile_pool(name="vsb", bufs=BL) as vsbp,
            tc.tile_pool(name="enc", bufs=ENC_BUFS) as encp,
            tc.tile_pool(name="ctx", bufs=2) as ctxp,
            tc.tile_pool(name="psA", bufs=3, space="PSUM") as psA,
            tc.tile_pool(name="psS", bufs=1, space="PSUM") as psS,
            tc.tile_pool(name="psT", bufs=1, space="PSUM") as psT,
            tc.tile_pool(name="psX", bufs=2, space="PSUM") as psX,
            tc.tile_pool(name="dram", bufs=1, space="DRAM") as dpool,
        ):
            ident = cpool.tile([128, 128], F32)
            make_identity(nc, ident[:])

            # ---------------- DMAs: LSTM critical path first ----------------
            wtp_cm = tc.tile_pool(name="wt", bufs=1)
            wtp = wtp_cm.__enter__()
            wta_sb = wtp.tile([128, 2, NK, HL], BF16)
            nc.sync.dma_start(wta_sb[:], wt_a[:])
            wtb_sb = wtp.tile([128, 2, NK, HL], BF16)
            nc.scalar.dma_start(wtb_sb[:], wt_b[:])
            xT_sb = wtp.tile([128, NKI, B], BF16)
            nc.sync.dma_start(xT_sb[:], xT[:])
            ahT_sb = wtp.tile([128, NKH, B], BF16)
            nc.sync.dma_start(ahT_sb[:], ahT[:])
            acT_sb = wtp.tile([HL, B], BF16)
            nc.sync.dma_start(acT_sb[:], acT[:])
            bias_sb = wtp.tile([HL, 4], F32)
            nc.sync.dma_start(bias_sb[:], bias[:])

            # enc stream: issue immediately behind wt_b on the scalar ring
            enc_sb = []
            for b in range(BL):
                et = encp.tile([128, NC_S, E], BF16, tag="enc")
                nc.scalar.dma_start(
                    et[:], enc[b].rearrange("p (r e) -> p r e", r=NC_S))
                enc_sb.append(et)

            # small constants on the sync ring (ahead of win/proc)
            wqT_sb = cpool.tile([HL, A], BF16)
            nc.sync.dma_start(wqT_sb[:], wqT[:])
            wcomb_sb = cpool.tile([128, A], BF16)
            nc.sync.dma_start(wcomb_sb[:], wcomb2[:])
            wo_sb = cpool.tile([A, 1], BF16)
            nc.sync.dma_start(wo_sb[:], wo_col[:])
            ca_sb = cpool.tile([A, 1], F32)
            nc.sync.dma_start(ca_sb[:], const_a[:])

            # ---------------- LSTM gates (H-shard, full batch) ----------------
            gate_sb = []
            for g in range(4):
                wsb = wta_sb if g < 2 else wtb_sb
                gg = g % 2
                ps = psA.tile([128, 512], F32, tag="mm")
                for k in range(NKI):
                    nc.tensor.matmul(ps[:, :B], wsb[:, gg, k, :], xT_sb[:, k, :],
                                     start=(k == 0), stop=False)
                for k in range(NKH):
                    nc.tensor.matmul(ps[:, :B], wsb[:, gg, NKI + k, :],
                                     ahT_sb[:, k, :],
                                     start=False, stop=(k == NKH - 1))
                sb = wtp.tile([HL, B], BF16, tag=f"gate{g}")
                fn = AF.Tanh if g == 2 else AF.Sigmoid
                nc.scalar.activation(sb[:], ps[:, :B], fn,
                                     bias=bias_sb[:, g:g + 1])
                gate_sb.append(sb)

            cT = wtp.tile([HL, B], BF16)
            nc.vector.tensor_mul(cT[:], gate_sb[1][:], acT_sb[:])
            tg = wtp.tile([HL, B], BF16)
            nc.vector.tensor_mul(tg[:], gate_sb[0][:], gate_sb[2][:])
            nc.vector.tensor_add(cT[:], cT[:], tg[:])
            nc.scalar.activation(tg[:], cT[:], AF.Tanh)
            hT_sh = wtp.tile([HL, B], BF16)
            nc.vector.tensor_mul(hT_sh[:], gate_sb[3][:], tg[:])

            # partial qry2 for the full batch: [B, A]
            ps_q = psA.tile([128, 512], F32, tag="mm")
            nc.tensor.matmul(ps_q[:, :A], hT_sh[:], wqT_sb[:],
                             start=True, stop=True)
            q_sb = wtp.tile([B, A], F32)
            nc.vector.tensor_copy(q_sb[:], ps_q[:, :A])

            qin = dpool.tile([B, A], F32)
            nc.gpsimd.dma_start(qin[:], q_sb[:])
            wtp_cm.__exit__(None, None, None)

            qout = dpool.tile([BL, A], F32)
            nc.gpsimd.collective_compute(
                "ReduceScatter",
                mybir.AluOpType.add,
                replica_groups=[list(range(NCORES))],
                ins=[qin[:].opt()],
                outs=[qout[:].opt()],
            )
            qg_sb = cpool.tile([BL, A], F32)
            nc.gpsimd.dma_start(qg_sb[:], qout[:])

            # ---------------- location features (pre-RS work) ----------------
            # v[b] = (W_loc-folded conv)(windows) in [A, S] layout, + proc.T
            v_sb = []
            for q in range(BL // 2):
                wq_t = winp.tile([128, S], BF16, tag="win")
                nc.sync.dma_start(wq_t[:], win2[q])
                for e2 in range(2):
                    b = 2 * q + e2
                    pt = procp.tile([A, S], BF16, tag="proc")
                    nc.sync.dma_start(pt[:], procT[b])
                    vt = vsbp.tile([A, S], BF16, tag="v")
                    base = 64 * e2
                    for c in range(2):
                        ps_v = psA.tile([128, 512], F32, tag="mm")
                        nc.tensor.matmul(
                            ps_v,
                            wcomb_sb[base:base + 64, :],
                            wq_t[base:base + 64, c * 512:(c + 1) * 512],
                            start=True, stop=True)
                        nc.vector.tensor_add(
                            vt[:, c * 512:(c + 1) * 512], ps_v,
                            pt[:, c * 512:(c + 1) * 512])
                    v_sb.append(vt)

            # ---------------- qry2 (post-RS) ----------------
            ps_t = psS.tile([128, 128], F32, tag="s")
            nc.tensor.transpose(ps_t[:, :BL], qg_sb[:], ident[:BL, :BL])
            qry2T = cpool.tile([A, BL], F32)
            nc.scalar.activation(qry2T[:], ps_t[:, :BL], AF.Identity,
                                 bias=ca_sb[:])

            # ---------------- scores ----------------
            # t = tanh(v + qry2[b]) ; scoresT[s-chunk, (c, b)] = t.T @ w_out
            scT_ps = psS.tile([128, 128], F32, tag="s")
            for b in range(BL):
                nc.scalar.activation(v_sb[b][:], v_sb[b][:], AF.Tanh,
                                     bias=qry2T[:, b:b + 1])
                for c in range(NC_S):
                    nc.tensor.matmul(
                        scT_ps[:, c * BL + b:c * BL + b + 1],
                        v_sb[b][:, c * 128:(c + 1) * 128],
                        wo_sb[:],
                        start=True, stop=True)
            scT_sb = cpool.tile([128, 128], F32)
            nc.vector.tensor_copy(scT_sb[:], scT_ps[:])

            # ---------------- softmax over S, all 16 examples ----------------
            sc_ps = psT.tile([BL, S], F32, tag="t")
            for c in range(NC_S):
                nc.tensor.transpose(sc_ps[:, c * 128:(c + 1) * 128],
                                    scT_sb[:, c * BL:(c + 1) * BL],
                                    ident[:])
            mx = cpool.tile([BL, 1], F32)
            nc.vector.reduce_max(mx[:], sc_ps[:], axis=mybir.AxisListType.X)
            nc.vector.tensor_scalar_mul(mx[:], mx[:], -1.0)
            sums = cpool.tile([BL, 1], F32)
            wts = cpool.tile([BL, S], F32)
            nc.scalar.activation(wts[:], sc_ps[:], AF.Exp, bias=mx[:],
                                 accum_out=sums[:])
            rs = cpool.tile([BL, 1], F32)
            nc.vector.reciprocal(rs[:], sums[:])
            nc.vector.tensor_scalar_mul(wts[:], wts[:], rs[:])

            wtsT = cpool.tile([128, NC_S, BL], BF16)
            for c in range(NC_S):
                ps_w = psS.tile([128, 128], F32, tag="s")
                nc.tensor.transpose(ps_w[:, :BL],
                                    wts[:, c * 128:(c + 1) * 128],
                                    ident[:BL, :BL])
                nc.vector.tensor_copy(wtsT[:, c, :], ps_w[:, :BL])

            # ---------------- context ----------------
            for grp in range(BL // 4):
                psx = psX.tile([128, 512], F32, tag="x")
                nc.vector.memset(psx[:], 0.0)
                for i in range(4):
                    b = 4 * grp + i
                    for c in range(NC_S):
                        nc.tensor.matmul(
                            psx[32 * i:32 * i + 1, :],
                            wtsT[:, c, b:b + 1],
                            enc_sb[b][:, c, :],
                            start=(c == 0), stop=(c == NC_S - 1),
                            tile_position=(0, 32 * i))
                ctx_sb = ctxp.tile([128, 512], F32, tag="ctx")
                nc.vector.tensor_copy(ctx_sb[:], psx[:])
                nc.sync.dma_start(out[4 * grp:4 * grp + 4, :],
                                  ctx_sb[0:128:32, :])

    nc.compile()
    return nc


_NC_CACHE = None


def _get_nc():
    global _NC_CACHE
    if _NC_CACHE is None:
        _NC_CACHE = build()
    return _NC_CACHE


def shard_inputs(prenet, prev_context, att_h, att_c, prev_weights, cum_weights,
                 enc_seq, proc_mem, mask, W_ih, W_hh, b_ih, b_hh, conv_w,
                 conv_b, W_loc, b_loc, W_q, b_q, W_out, b_out, **_unused):
    f32 = np.float32
    c = np.ascontiguousarray

    W_ih4 = np.asarray(W_ih, f32).reshape(4, H, PE_DIM)
    W_hh4 = np.asarray(W_hh, f32).reshape(4, H, H)
    bias4 = (np.asarray(b_ih, f32) + np.asarray(b_hh, f32)).reshape(4, H)

    x = np.concatenate([np.asarray(prenet, f32),
                        np.asarray(prev_context, f32)], axis=1)  # [B, 768]
    xT_h = c(x.T.reshape(NKI, 128, B).transpose(1, 0, 2).astype(BF))
    ahT_h = c(np.asarray(att_h, f32).T.reshape(NKH, 128, B)
              .transpose(1, 0, 2).astype(BF))

    # W_loc folded into the conv kernel: wcomb[(c,k), a]
    cw = np.asarray(conv_w, f32).reshape(F, TAPS)          # [F, 62]
    wcomb = (np.asarray(W_loc, f32) @ cw).T                # [62, A]
    wcomb2_h = np.zeros((128, A), f32)
    wcomb2_h[0:TAPS] = wcomb
    wcomb2_h[64:64 + TAPS] = wcomb
    wcomb2_h = c(wcomb2_h.astype(BF))

    # constant additive term for the tanh argument (per A)
    const = (np.asarray(b_q, f32) + np.asarray(b_loc, f32)
             + np.asarray(W_loc, f32) @ np.asarray(conv_b, f32))  # [A]
    const_h = c(const.reshape(A, 1))
    wo_h = c(np.asarray(W_out, f32).reshape(A, 1).astype(BF))

    # conv windows (padded), per example: [62, S]
    cum = np.asarray(cum_weights, f32)
    prv = np.asarray(prev_weights, f32)
    padded = np.zeros((B, 2, KW // 2 + S + KW // 2 + 1), f32)
    padded[:, 0, KW // 2:KW // 2 + S] = cum
    padded[:, 1, KW // 2:KW // 2 + S] = prv
    sw = np.lib.stride_tricks.sliding_window_view(
        padded, S, axis=2)                                  # [B, 2, KW+1, S]
    win = sw[:, :, :KW, :].reshape(B, TAPS, S)              # [B, 62, S]

    enc_bf = np.asarray(enc_seq, f32).reshape(B, NC_S, 128, E) \
        .transpose(0, 2, 1, 3).reshape(B, 128, NC_S * E).astype(BF)
    procT_bf = np.asarray(proc_mem, f32).transpose(0, 2, 1).astype(BF)

    in_maps = []
    for j in range(NCORES):
        bj = slice(BL * j, BL * (j + 1))
        hj = slice(HL * j, HL * (j + 1))

        wt = np.concatenate(
            [W_ih4[:, hj].reshape(4, HL, NKI, 128),
             W_hh4[:, hj].reshape(4, HL, NKH, 128)], axis=2)  # [4, HL, 14, 128]
        wt = wt.transpose(3, 0, 2, 1).astype(BF)              # [128, 4, 14, HL]

        win_j = win[bj]                                       # [16, 62, S]
        win2_h = np.zeros((BL // 2, 128, S), f32)
        win2_h[:, 0:TAPS] = win_j[0::2]
        win2_h[:, 64:64 + TAPS] = win_j[1::2]

        in_maps.append({
            "wt_a": c(wt[:, 0:2]),
            "wt_b": c(wt[:, 2:4]),
            "xT": xT_h,
            "ahT": ahT_h,
            "acT": c(np.asarray(att_c, f32)[:, hj].T.astype(BF)),
            "bias": c(bias4[:, hj].T),
            "wqT": c(np.asarray(W_q, f32)[:, hj].T.astype(BF)),
            "const_a": const_h,
            "wo_col": wo_h,
            "wcomb2": wcomb2_h,
            "win2": c(win2_h.astype(BF)),
            "procT": c(procT_bf[bj]),
            "enc": c(enc_bf[bj]),
        })
    return in_maps


def kernel(**inputs):
    assert not np.any(np.asarray(inputs["mask"])), \
        "kernel assumes mask == 0 (softmax-shift support not implemented)"
    nc = _get_nc()
    in_maps = shard_inputs(**inputs)
    res = run_bass_kernel_spmd(nc, in_maps, core_ids=list(range(NCORES)))
    return np.concatenate([res.results[j]["out"] for j in range(NCORES)],
                          axis=0)


if __name__ == "__main__":
    print("building...")
    _get_nc()
    print("built ok")
